# revision 1
# baseline (speedup 1.0000x reference)
"""FISTA encoder v2: exact-class fp16 3-term matmuls + fused custom shrink.

A@x decomposed as Ah@xh + Ah@xl + Al@xh with fp16 Dekker-style pairs
(dropped Al@xl ~ 2^-22). Static Y folded into the contraction with a
3-way fp16 split (Yh/Ym/Yl) riding existing matmul slots, so the static
DtY part is exact to ~2^-33. x is kept fp32 only transiently per group;
matmul state is the (xh, xl) fp16 pair. u_n = A@x_n + DtY accumulates in
PSUM; z_n = (1+tt)*u_n - v_{n-1}; x_{n+1} = softshrink(z_n) in one fused
DVE op; v_n = tt_n*u_n saved by ACT.
"""
import sys
if '/opt/trn_rl_repo' not in sys.path:
    sys.path.insert(0, '/opt/trn_rl_repo')
import numpy as np

# ---- custom fused DVE op: x = softshrink(C0*u - v, lambd) ----------------
def _register_shrink_op():
    from concourse.dve_ops import OPS, DveOp
    from concourse import dve_ops as _d
    from concourse.dve_spec import Spec, Src0, Src1, C0, C1, Zero, maxx, minn
    for op in OPS:
        if op.name == "FISTA_SHRINK":
            return op
    _q = Src0 * C0 - Src1
    _c = minn(maxx(_q, Zero - C1), C1)

    def _ref(in0, in1, s0, s1, imm2):
        q = in0 * s0 - in1
        return q - np.clip(q, -s1, s1)

    op = DveOp("FISTA_SHRINK", Spec(body=_q - _c, reference=_ref),
               subdim=False, uops_sha={})
    OPS.append(op)
    _d._SUB_OPCODE_FOR_NAME[op.name] = _d._CUSTOM_DVE_ROW_BASE + len(OPS) - 1
    _d.CUSTOM_DVE_SPECS[op.name] = op.spec
    for ver in ("v3", "v4"):
        try:
            op.compile(ver)
        except ValueError as e:
            got = str(e).split(f"{ver}: ")[1].split(" ")[0]
            op.uops_sha[ver] = got
            op.compile(ver)
    return op


def _shrink(nc, out, u, v, c0, lambd):
    op = _register_shrink_op()
    return nc.vector._custom_dve(op, out=out, in0=u, in1=v,
                                 s0=float(c0), s1=float(lambd))


T = 36
MAXITER = 100
LAMBD0 = 0.01
N_CORES = 8
B, P, K = 4, 16384, 161
PLOC = P // N_CORES
F = B * PLOC                 # 8192 columns per core
NG = 8
FG = F // NG                 # 1024 columns per group

_CACHE = {}


def _host_constants(Drr, Dtheta):
    Drr = np.asarray(Drr, np.float32)
    Dtheta = np.asarray(Dtheta, np.float32)
    i = np.arange(T, dtype=np.float32)
    powr = (Drr[None, :] ** i[:, None]).astype(np.float32)
    sign = np.where(i[:, None] % 2 == 0, np.float32(1.0), np.float32(-1.0))
    ang = (i[:, None] * Dtheta[None, :]).astype(np.float32)
    cosm = np.cos(ang).astype(np.float32)
    sinm = np.sin(ang).astype(np.float32)
    ones = np.ones((T, 1), np.float32)
    dic = np.concatenate(
        [ones, powr * cosm, sign * powr * cosm, powr * sinm, sign * powr * sinm],
        axis=1).astype(np.float32)
    G = np.sqrt(np.sum(dic * dic, axis=0, dtype=np.float32)).astype(np.float32)
    G = np.where(G == 0, np.sqrt(np.float32(T)), G).astype(np.float32)
    D = (dic / G).astype(np.float32)
    DtD = (D.T @ D).astype(np.float32)
    L = np.sqrt(np.sum(DtD.astype(np.float64) ** 2)).astype(np.float32)
    linv = np.float32(1.0) / L
    A = (np.eye(K, dtype=np.float32) - DtD * linv).astype(np.float32)
    lambd = np.float32(LAMBD0 * linv)
    W = np.concatenate([A, (D * linv).astype(np.float32)], axis=0)  # (197,161)

    Wh = W.astype(np.float16)
    Wl = (W - Wh.astype(np.float32)).astype(np.float16)
    wh1 = np.ascontiguousarray(Wh[0:128])          # (128,161) x_hi rows, hi part
    wl1 = np.ascontiguousarray(Wl[0:128])
    # S_A rows: [xh_lo(33); Yh(36); Ym(36)] -> T1 weights [Ah_lo; Dh; Dh]
    wa1 = np.concatenate([Wh[128:161], Wh[161:197], Wh[161:197]], axis=0)  # (105,161)
    # S_A x T3 weights [Al_lo; Dl; Dl]
    wa2 = np.concatenate([Wl[128:161], Wl[161:197], Wl[161:197]], axis=0)  # (105,161)
    # S_B rows: [xl_lo(33); Yl(36)] -> weights [Ah_lo; Dh]
    wb = np.concatenate([Wh[128:161], Wh[161:197]], axis=0)                # (69,161)

    tts = []
    t = np.float32(1.0)
    for _ in range(MAXITER):
        t_new = (np.float32(1.0) + np.sqrt(np.float32(1.0) + np.float32(4.0) * t * t)) / np.float32(2.0)
        tts.append(np.float32((t - np.float32(1.0)) / t_new))
        t = t_new
    return dict(wh1=wh1, wl1=wl1, wa1=wa1, wa2=wa2, wb=wb,
                lambd=lambd, tts=tts)


def _build_bass(lambd, tts, xl_engine="gpsimd"):
    import concourse.bass as bass
    import concourse.tile as tile
    from concourse import bacc, mybir
    dt = mybir.dt

    nc = bacc.Bacc("TRN2", target_bir_lowering=False, debug=False,
                   num_devices=N_CORES)
    wh1_d = nc.dram_tensor("wh1", [128, K], dt.float16, kind="ExternalInput").ap()
    wl1_d = nc.dram_tensor("wl1", [128, K], dt.float16, kind="ExternalInput").ap()
    wa1_d = nc.dram_tensor("wa1", [105, K], dt.float16, kind="ExternalInput").ap()
    wa2_d = nc.dram_tensor("wa2", [105, K], dt.float16, kind="ExternalInput").ap()
    wb_d = nc.dram_tensor("wb", [69, K], dt.float16, kind="ExternalInput").ap()
    yh_d = nc.dram_tensor("yh", [T, F], dt.float16, kind="ExternalInput").ap()
    ym_d = nc.dram_tensor("ym", [T, F], dt.float16, kind="ExternalInput").ap()
    yl_d = nc.dram_tensor("yl", [T, F], dt.float16, kind="ExternalInput").ap()
    xout_d = nc.dram_tensor("xout", [K, F], dt.float32, kind="ExternalOutput").ap()

    lam = float(lambd)

    with tile.TileContext(nc) as tc:
        with tc.tile_pool(name="wp", bufs=1) as wp, \
             tc.tile_pool(name="state", bufs=1) as state, \
             tc.tile_pool(name="xt", bufs=3) as xtp, \
             tc.tile_pool(name="ph", bufs=2, space="PSUM") as php, \
             tc.tile_pool(name="pl", bufs=2, space="PSUM") as plp:

            wh1 = wp.tile([128, K], dt.float16, tag="wh1")
            wl1 = wp.tile([128, K], dt.float16, tag="wl1")
            wa1 = wp.tile([105, K], dt.float16, tag="wa1")
            wa2 = wp.tile([105, K], dt.float16, tag="wa2")
            wb = wp.tile([69, K], dt.float16, tag="wb")
            for t_, d_ in ((wh1, wh1_d), (wl1, wl1_d), (wa1, wa1_d),
                           (wa2, wa2_d), (wb, wb_d)):
                nc.sync.dma_start(t_[:], d_[:])

            XH1, XL1, SA, SB, V1, V2 = [], [], [], [], [], []
            for g in range(NG):
                cols = slice(g * FG, (g + 1) * FG)
                xh1 = state.tile([128, FG], dt.float16, tag=f"xh1_{g}")
                xl1 = state.tile([128, FG], dt.float16, tag=f"xl1_{g}")
                sa = state.tile([105, FG], dt.float16, tag=f"sa_{g}")
                sb_ = state.tile([69, FG], dt.float16, tag=f"sb_{g}")
                v1 = state.tile([128, FG], dt.float32, tag=f"v1_{g}")
                v2 = state.tile([33, FG], dt.float32, tag=f"v2_{g}")
                nc.vector.memset(xh1[:], 0.0)
                nc.vector.memset(xl1[:], 0.0)
                nc.vector.memset(sa[0:33, :], 0.0)
                nc.vector.memset(sb_[0:33, :], 0.0)
                nc.vector.memset(v1[:], 0.0)
                nc.vector.memset(v2[:], 0.0)
                nc.sync.dma_start(sa[33:69, :], yh_d[:, cols])
                nc.sync.dma_start(sa[69:105, :], ym_d[:, cols])
                nc.sync.dma_start(sb_[33:69, :], yl_d[:, cols])
                XH1.append(xh1); XL1.append(xl1); SA.append(sa); SB.append(sb_)
                V1.append(v1); V2.append(v2)

            xl_eng = nc.gpsimd if xl_engine == "gpsimd" else nc.vector

            for n in range(MAXITER):
                c0 = float(np.float32(1.0) + (tts[n - 1] if n > 0 else np.float32(0.0)))
                sv = float(tts[n])
                last = (n == MAXITER - 1)
                for g in range(NG):
                    xh1, xl1, sa, sb_ = XH1[g], XL1[g], SA[g], SB[g]
                    v1, v2 = V1[g], V2[g]
                    ph = php.tile([128, FG], dt.float32, tag="ph")
                    pl = plp.tile([33, FG], dt.float32, tag="pl")
                    # weight-major order: each weight block loads once and
                    # serves both 512-col halves before switching.
                    if n == 0:
                        mm_list = [(wa1, sa), (wb, sb_), (wa2, sa)]
                    else:
                        mm_list = [(wh1, xh1), (wh1, xl1), (wl1, xh1),
                                   (wa1, sa), (wb, sb_), (wa2, sa)]
                    nmm = len(mm_list)
                    for pt, wlo, whi in ((ph, 0, 128), (pl, 128, K)):
                        wc = slice(wlo, whi)
                        for mi, (wt_, rt_) in enumerate(mm_list):
                            for h in range(FG // 512):
                                s = slice(h * 512, (h + 1) * 512)
                                nc.tensor.matmul(pt[:, s], wt_[:, wc], rt_[:, s],
                                                 start=(mi == 0),
                                                 stop=(mi == nmm - 1))
                    # x fp32, transient
                    x1t = xtp.tile([128, FG], dt.float32, tag="x1t")
                    x2t = xtp.tile([33, FG], dt.float32, tag="x2t")
                    _shrink(nc, x1t[:], ph[:], v1[:], c0, lam)
                    _shrink(nc, x2t[:], pl[:], v2[:], c0, lam)
                    if not last:
                        nc.scalar.mul(v1[:], ph[:], sv)
                        nc.scalar.mul(v2[:], pl[:], sv)
                        # xh = fp16(x); xl = x - xh
                        nc.scalar.copy(xh1[:], x1t[:])
                        nc.scalar.copy(sa[0:33, :], x2t[:])
                        xl_eng.tensor_sub(xl1[:], x1t[:], xh1[:])
                        xl_eng.tensor_sub(sb_[0:33, :], x2t[:], sa[0:33, :])
                    else:
                        cols = slice(g * FG, (g + 1) * FG)
                        nc.sync.dma_start(xout_d[0:128, cols], x1t[:])
                        nc.sync.dma_start(xout_d[128:K, cols], x2t[:])

    nc.compile()
    return nc


def _get_compiled(Drr, Dtheta):
    key = (np.asarray(Drr, np.float32).tobytes(),
           np.asarray(Dtheta, np.float32).tobytes())
    if key not in _CACHE:
        hc = _host_constants(Drr, Dtheta)
        nc = _build_bass(hc["lambd"], hc["tts"])
        _CACHE[key] = (nc, hc)
    return _CACHE[key]


def kernel(x, Drr, Dtheta):
    from concourse.bass_utils import run_bass_kernel_spmd
    x = np.asarray(x, np.float32)
    nc, hc = _get_compiled(Drr, Dtheta)

    in_maps = []
    for c in range(N_CORES):
        xs = x[:, :, c * PLOC:(c + 1) * PLOC]
        yfull = np.ascontiguousarray(xs.transpose(1, 0, 2).reshape(T, F))
        yh = yfull.astype(np.float16)
        ym = (yfull - yh.astype(np.float32)).astype(np.float16)
        yl = (yfull - yh.astype(np.float32) - ym.astype(np.float32)).astype(np.float16)
        in_maps.append({"wh1": hc["wh1"], "wl1": hc["wl1"], "wa1": hc["wa1"],
                        "wa2": hc["wa2"], "wb": hc["wb"],
                        "yh": yh, "ym": ym, "yl": yl})

    res = run_bass_kernel_spmd(nc, in_maps, core_ids=list(range(N_CORES)))
    global LAST_RESULTS
    LAST_RESULTS = res
    out = np.empty((B, K, P), np.float32)
    for c in range(N_CORES):
        xo = res.results[c]["xout"]
        out[:, :, c * PLOC:(c + 1) * PLOC] = (
            xo.reshape(K, B, PLOC).transpose(1, 0, 2))
    return out



# revision 8
# speedup vs baseline: 1.8495x; 1.8495x over previous
"""FISTA encoder v3: UD-direct delta formulation.

Per iteration only ONE fp16 matmul pass over the (small) dx increment plus a
rank-36 residual path, with all history kept in fp32 SBUF states:

  z_n   = xq_n + [ (tt*I + (1+tt)*M) @ dxq_n  +  Uh @ mh_{n-1} ]   (psum)
  dxq_{n+1} = fp16( softshrink(z_n, lambd) - xq_n )                 (DVE custom)
  xq_{n+1}  = xq_n + dxq_{n+1}                                       (fp32 add)
  macc-psum = I @ mh + D @ dxq   ->  mh_n = fp16(macc)               (ACT round)

where M = -(1/L) D^T D, U = -(1/L) D^T, mh tracks fp16(D@x - Y), and xq is the
accumulated-quantized-dx lattice (error feedback keeps fp16 noise bounded).
W-blocks (tt*A16 + M16) are rebuilt on-device each iteration (tts are static).

Matmul layout per 512-col slab (4 instrs):
  out t1 [92p]  = z rows 0:92      <- chunks c1 [128p] = [dx 0:92 ; mh 36]
  out t2 [128p] = [z 92:161 | pad | macc 36]  <- c1, c2 [69p] = dx 92:161
"""
import sys
if '/opt/trn_rl_repo' not in sys.path:
    sys.path.insert(0, '/opt/trn_rl_repo')
import numpy as np

T = 36
MAXITER = 100
LAMBD0 = 0.01
N_CORES = 8
B, P, K = 4, 16384, 161
PLOC = P // N_CORES
F = B * PLOC                 # 8192 columns per core
NG = 8
FG = F // NG                 # 1024 columns per group
KA, KB = 128, 33             # row split of K=161 (32-aligned bases)

# engine/layout knobs
OPB_MODE = "split"           # "split" | "gp_group"
OPB_DVE_GROUPS = 5           # groups whose [92p] xq-update runs on DVE (rest GP)
REBUILD_ENG = "dve"          # stt not supported on gpsimd/Pool

_CACHE = {}


def _register_op(name, body_builder, ref):
    from concourse.dve_ops import OPS, DveOp
    from concourse import dve_ops as _d
    from concourse.dve_spec import Spec
    for op in OPS:
        if op.name == name:
            return op
    op = DveOp(name, Spec(body=body_builder(), reference=ref),
               subdim=False, uops_sha={})
    OPS.append(op)
    _d._SUB_OPCODE_FOR_NAME[name] = _d._CUSTOM_DVE_ROW_BASE + len(OPS) - 1
    _d.CUSTOM_DVE_SPECS[name] = op.spec
    for ver in ("v3", "v4"):
        try:
            op.compile(ver)
        except ValueError as e:
            got = str(e).split(f"{ver}: ")[1].split(" ")[0]
            op.uops_sha[ver] = got
            op.compile(ver)
    return op


def _op_shrinkdx():
    # out = softshrink(in0 + in1, C1) - in0
    from concourse.dve_spec import Src0, Src1, C1, Zero, maxx, minn

    def body():
        z = Src0 + Src1
        c = minn(maxx(z, Zero - C1), C1)
        return (z - c) - Src0

    def ref(in0, in1, s0, s1, imm2):
        z = in0 + in1
        return (z - np.clip(z, -s1, s1)) - in0

    return _register_op("SHRINKDX", body, ref)


def _op_shrinkx():
    # out = softshrink(in0 + in1, C1)
    from concourse.dve_spec import Src0, Src1, C1, Zero, maxx, minn

    def body():
        z = Src0 + Src1
        c = minn(maxx(z, Zero - C1), C1)
        return z - c

    def ref(in0, in1, s0, s1, imm2):
        z = in0 + in1
        return z - np.clip(z, -s1, s1)

    return _register_op("SHRINKX", body, ref)


def _host_constants(Drr, Dtheta):
    Drr = np.asarray(Drr, np.float32)
    Dtheta = np.asarray(Dtheta, np.float32)
    i = np.arange(T, dtype=np.float32)
    powr = (Drr[None, :] ** i[:, None]).astype(np.float32)
    sign = np.where(i[:, None] % 2 == 0, np.float32(1.0), np.float32(-1.0))
    ang = (i[:, None] * Dtheta[None, :]).astype(np.float32)
    cosm = np.cos(ang).astype(np.float32)
    sinm = np.sin(ang).astype(np.float32)
    ones = np.ones((T, 1), np.float32)
    dic = np.concatenate(
        [ones, powr * cosm, sign * powr * cosm, powr * sinm, sign * powr * sinm],
        axis=1).astype(np.float32)
    G = np.sqrt(np.sum(dic * dic, axis=0, dtype=np.float32)).astype(np.float32)
    G = np.where(G == 0, np.sqrt(np.float32(T)), G).astype(np.float32)
    D = (dic / G).astype(np.float32)                      # (36,161)
    DtD = (D.T @ D).astype(np.float32)
    L = np.sqrt(np.sum(DtD.astype(np.float64) ** 2)).astype(np.float32)
    linv = np.float32(1.0) / L
    lambd = np.float32(LAMBD0 * linv)

    U = (-linv * D.T).astype(np.float32)                  # (161,36)
    M = (U @ D).astype(np.float32)                        # (161,161)
    A = (np.eye(K, dtype=np.float32) + M).astype(np.float32)
    f16 = lambda a: a.astype(np.float16)
    Uh = f16(U)
    Dq = f16(D)
    M16 = f16(M)
    A16 = f16(A)

    # --- static weight tiles (fp16), [contract_rows, out_cols] ---
    # Partition bases of compute accesses must be 32-aligned, so:
    # c1 chunk [128p]  = dx rows 0:128
    # c2 chunk [100p]  = [dx 128:161 (0:33) | gap (33:64) | mh (64:100)]
    # t1 psum  [128p]  = z rows 0:128
    # t2 psum  [100p]  = [z 128:161 (0:33) | gap | macc (64:100)]
    I36 = np.eye(T, dtype=np.float16)

    # W-regions are zero here (rebuilt on device); static rows filled.
    wc1t1 = np.zeros((128, KA), np.float16)               # rebuilt fully
    wc1t2 = np.zeros((128, 100), np.float16)
    wc1t2[:, 64:100] = Dq[:, 0:KA].T                      # dx 0:128 -> macc
    wc2t1 = np.zeros((100, KA), np.float16)
    wc2t1[64:100, :] = Uh[0:KA, :].T                      # mh -> z 0:128
    wc2t2 = np.zeros((100, 100), np.float16)
    wc2t2[0:33, 64:100] = Dq[:, KA:K].T                   # dx 128:161 -> macc
    wc2t2[64:100, 0:33] = Uh[KA:K, :].T                   # mh -> z 128:161
    wc2t2[64:100, 64:100] = I36                           # mh -> macc

    # A16/M16 source blocks for the per-iteration W rebuild
    a_c1t1 = np.ascontiguousarray(A16[0:KA, 0:KA])        # (128,128)
    m_c1t1 = np.ascontiguousarray(M16[0:KA, 0:KA])
    a_c1t2 = np.ascontiguousarray(A16[0:KA, KA:K])        # (128,33)
    m_c1t2 = np.ascontiguousarray(M16[0:KA, KA:K])
    a_c2t1 = np.ascontiguousarray(A16[KA:K, 0:KA])        # (33,128)
    m_c2t1 = np.ascontiguousarray(M16[KA:K, 0:KA])
    a_c2t2 = np.ascontiguousarray(A16[KA:K, KA:K])        # (33,33)
    m_c2t2 = np.ascontiguousarray(M16[KA:K, KA:K])

    tts = []
    t = np.float32(1.0)
    for _ in range(MAXITER):
        t_new = (np.float32(1.0) + np.sqrt(np.float32(1.0) + np.float32(4.0) * t * t)) / np.float32(2.0)
        tts.append(np.float32((t - np.float32(1.0)) / t_new))
        t = t_new
    return dict(lambd=lambd, tts=tts,
                wc1t1=wc1t1, wc1t2=wc1t2, wc2t1=wc2t1, wc2t2=wc2t2,
                a_c1t1=a_c1t1, m_c1t1=m_c1t1, a_c1t2=a_c1t2, m_c1t2=m_c1t2,
                a_c2t1=a_c2t1, m_c2t1=m_c2t1, a_c2t2=a_c2t2, m_c2t2=m_c2t2)


def _build_bass(lambd, tts):
    import concourse.bass as bass
    import concourse.tile as tile
    from concourse import bacc, mybir
    dt = mybir.dt
    alu = mybir.AluOpType

    shdx = _op_shrinkdx()
    shx = _op_shrinkx()

    nc = bacc.Bacc("TRN2", target_bir_lowering=False, debug=False,
                   num_devices=N_CORES)
    wd = {}
    for name, shp in (("wc1t1", [128, KA]), ("wc1t2", [128, 100]),
                      ("wc2t1", [100, KA]), ("wc2t2", [100, 100]),
                      ("a_c1t1", [KA, KA]), ("m_c1t1", [KA, KA]),
                      ("a_c1t2", [KA, KB]), ("m_c1t2", [KA, KB]),
                      ("a_c2t1", [KB, KA]), ("m_c2t1", [KB, KA]),
                      ("a_c2t2", [KB, KB]), ("m_c2t2", [KB, KB])):
        wd[name] = nc.dram_tensor(name, shp, dt.float16, kind="ExternalInput").ap()
    ymh_d = nc.dram_tensor("ymh", [T, F], dt.float16, kind="ExternalInput").ap()
    xout_d = nc.dram_tensor("xout", [K, F], dt.float32, kind="ExternalOutput").ap()

    lam = float(lambd)

    with tile.TileContext(nc) as tc:
        with tc.tile_pool(name="stat", bufs=1) as stat, \
             tc.tile_pool(name="wb", bufs=2) as wbp, \
             tc.tile_pool(name="xt", bufs=2) as xtp, \
             tc.tile_pool(name="p1", bufs=2, space="PSUM") as p1p, \
             tc.tile_pool(name="p2", bufs=2, space="PSUM") as p2p:

            # static tiles
            st = {}
            for name, shp in (("a_c1t1", [KA, KA]), ("m_c1t1", [KA, KA]),
                              ("a_c1t2", [KA, KB]), ("m_c1t2", [KA, KB]),
                              ("a_c2t1", [KB, KA]), ("m_c2t1", [KB, KA]),
                              ("a_c2t2", [KB, KB]), ("m_c2t2", [KB, KB])):
                st[name] = stat.tile(shp, dt.float16, tag=name, name=name)
                nc.sync.dma_start(st[name][:], wd[name][:])

            # rhs state tiles
            c1 = stat.tile([128, F], dt.float16, tag="c1", name="c1")
            c2 = stat.tile([100, F], dt.float16, tag="c2", name="c2")
            xqa = stat.tile([KA, F], dt.float32, tag="xqa", name="xqa")
            xqb = stat.tile([KB, F], dt.float32, tag="xqb", name="xqb")
            nc.vector.memset(c1[:], 0.0)
            nc.vector.memset(c2[:], 0.0)       # incl. gap partitions 33:64
            nc.vector.memset(xqa[:], 0.0)
            nc.vector.memset(xqb[:], 0.0)
            nc.sync.dma_start(c2[64:100, :], ymh_d[:])       # mh_0 = fp16(-Y)

            # weight tiles: double-buffered via pool bufs=2; static regions
            # must be present in BOTH buffers.
            wtiles = []
            for b in range(2):
                ws = {}
                for name, shp in (("wc1t1", [128, KA]), ("wc1t2", [128, 100]),
                                  ("wc2t1", [100, KA]), ("wc2t2", [100, 100])):
                    w = wbp.tile(shp, dt.float16, tag=name, name=name)
                    nc.sync.dma_start(w[:], wd[name][:])
                    ws[name] = w
                wtiles.append(ws)

            reb_eng = nc.gpsimd if REBUILD_ENG == "gp" else nc.vector

            for n in range(MAXITER):
                tt = float(tts[n - 1]) if n > 0 else 0.0
                last = (n == MAXITER - 1)
                ws = wtiles[n % 2]
                # rebuild W-blocks: W = tt*A16 + M16  (dx-row regions only)
                reb_eng.scalar_tensor_tensor(
                    ws["wc1t1"][:, :], st["a_c1t1"][:], tt, st["m_c1t1"][:],
                    alu.mult, alu.add)
                reb_eng.scalar_tensor_tensor(
                    ws["wc1t2"][:, 0:KB], st["a_c1t2"][:], tt, st["m_c1t2"][:],
                    alu.mult, alu.add)
                reb_eng.scalar_tensor_tensor(
                    ws["wc2t1"][0:KB, :], st["a_c2t1"][:], tt, st["m_c2t1"][:],
                    alu.mult, alu.add)
                reb_eng.scalar_tensor_tensor(
                    ws["wc2t2"][0:KB, 0:KB], st["a_c2t2"][:], tt, st["m_c2t2"][:],
                    alu.mult, alu.add)

                for g in range(NG):
                    cols = slice(g * FG, (g + 1) * FG)
                    p1 = p1p.tile([KA, FG], dt.float32, tag="p1", name="p1")
                    p2 = p2p.tile([100, FG], dt.float32, tag="p2", name="p2")
                    for h in range(FG // 512):
                        s = slice(h * 512, (h + 1) * 512)
                        gs = slice(g * FG + h * 512, g * FG + (h + 1) * 512)
                        nc.tensor.matmul(p1[:, s], ws["wc1t1"][:, :], c1[:, gs],
                                         start=True, stop=False)
                        nc.tensor.matmul(p1[:, s], ws["wc2t1"][:, :], c2[:, gs],
                                         start=False, stop=True)
                        nc.tensor.matmul(p2[:, s], ws["wc1t2"][:, :], c1[:, gs],
                                         start=True, stop=False)
                        nc.tensor.matmul(p2[:, s], ws["wc2t2"][:, :], c2[:, gs],
                                         start=False, stop=True)
                    if not last:
                        # dxq pieces (fp16) into rhs tiles
                        nc.vector._custom_dve(shdx, out=c1[:, cols],
                                              in0=xqa[:, cols], in1=p1[:, :],
                                              s1=lam)
                        nc.vector._custom_dve(shdx, out=c2[0:KB, cols],
                                              in0=xqb[:, cols], in1=p2[0:KB, :],
                                              s1=lam)
                        # mh round: macc psum partitions 64:100 -> c2[64:100]
                        nc.scalar.copy(c2[64:100, cols], p2[64:100, :])
                        # xq += dxq, split across DVE/GPSIMD by measured rates
                        if OPB_MODE == "gp_group":
                            ea = eb = nc.gpsimd
                        else:
                            ea = nc.vector if g < OPB_DVE_GROUPS else nc.gpsimd
                            eb = nc.gpsimd
                        ea.tensor_tensor(xqa[:, cols], xqa[:, cols],
                                         c1[:, cols], alu.add)
                        eb.tensor_tensor(xqb[:, cols], xqb[:, cols],
                                         c2[0:KB, cols], alu.add)
                    else:
                        xa = xtp.tile([KA, FG], dt.float32, tag="xa")
                        xb = xtp.tile([KB, FG], dt.float32, tag="xb")
                        nc.vector._custom_dve(shx, out=xa[:], in0=xqa[:, cols],
                                              in1=p1[:, :], s1=lam)
                        nc.vector._custom_dve(shx, out=xb[:], in0=xqb[:, cols],
                                              in1=p2[0:KB, :], s1=lam)
                        nc.sync.dma_start(xout_d[0:KA, cols], xa[:])
                        nc.sync.dma_start(xout_d[KA:K, cols], xb[:])
    nc.compile()
    return nc


def _get_compiled(Drr, Dtheta):
    key = (np.asarray(Drr, np.float32).tobytes(),
           np.asarray(Dtheta, np.float32).tobytes())
    if key not in _CACHE:
        hc = _host_constants(Drr, Dtheta)
        nc = _build_bass(hc["lambd"], hc["tts"])
        _CACHE[key] = (nc, hc)
    return _CACHE[key]


def kernel(x, Drr, Dtheta):
    from concourse.bass_utils import run_bass_kernel_spmd
    x = np.asarray(x, np.float32)
    nc, hc = _get_compiled(Drr, Dtheta)

    wkeys = ["wc1t1", "wc1t2", "wc2t1", "wc2t2",
             "a_c1t1", "m_c1t1", "a_c1t2", "m_c1t2",
             "a_c2t1", "m_c2t1", "a_c2t2", "m_c2t2"]
    in_maps = []
    for c in range(N_CORES):
        xs = x[:, :, c * PLOC:(c + 1) * PLOC]
        yfull = np.ascontiguousarray(xs.transpose(1, 0, 2).reshape(T, F))
        ymh = (-yfull).astype(np.float16)
        m = {k: hc[k] for k in wkeys}
        m["ymh"] = ymh
        in_maps.append(m)

    res = run_bass_kernel_spmd(nc, in_maps, core_ids=list(range(N_CORES)))
    global LAST_RESULTS
    LAST_RESULTS = res
    out = np.empty((B, K, P), np.float32)
    for c in range(N_CORES):
        xo = res.results[c]["xout"]
        out[:, :, c * PLOC:(c + 1) * PLOC] = (
            xo.reshape(K, B, PLOC).transpose(1, 0, 2))
    return out


# revision 9
# speedup vs baseline: 1.8508x; 1.0007x over previous
"""FISTA encoder v4: UD-direct delta formulation + 2-iteration cadence.

Math (per iteration, all on-device):
  z_n = B + psum[ W-blocks @ q-increments + Uh @ mh ]
  o_n = fp16( softshrink(z_n, lambd) - B )     (fused DVE op, writes the
                                                next matmul rhs directly)
  mh  = fp16( psum-macc )                       (ACT round; macc = I@mh + D@dq)
  B  += o_n  every 2nd iteration               (fp32 lattice base, GPSIMD)

where the fp32 base B is the accumulated sum of fp16-quantized increments
(error feedback keeps quantization noise from integrating through FISTA's
momentum recursion), mh tracks fp16(D@x - Y) in a 36-row side channel kept
inside the matmul via identity weight columns, and the A-matrix is applied in
factored form (identity exact via the increment path; M = -(1/L)D^T D in fp16
with per-iteration weights tt*I + (1+tt)*M precomputed on host and DMA'd).
Odd iterations contract only the newest increment; even (fresh-base)
iterations contract the two previous increments with +/- weight sets.

Per-iteration engine budget (measured): tensor 96 matmul/iter avg,
DVE 16 fused-shrink subtiles, GPSIMD 8 base-update subtiles avg, ACT 8
mh-round subtiles; weight tiles stream from DRAM (~0.1 ms total).
"""
import sys
if '/opt/trn_rl_repo' not in sys.path:
    sys.path.insert(0, '/opt/trn_rl_repo')
import numpy as np

T = 36
MAXITER = 100
LAMBD0 = 0.01
N_CORES = 8
B, P, K = 4, 16384, 161
PLOC = P // N_CORES
F = B * PLOC                 # 8192 columns per core
NG = 8
FG = F // NG                 # 1024 columns per group

def _register_op(name, body_builder, ref):
    from concourse.dve_ops import OPS, DveOp
    from concourse import dve_ops as _d
    from concourse.dve_spec import Spec
    for op in OPS:
        if op.name == name:
            return op
    op = DveOp(name, Spec(body=body_builder(), reference=ref),
               subdim=False, uops_sha={})
    OPS.append(op)
    _d._SUB_OPCODE_FOR_NAME[name] = _d._CUSTOM_DVE_ROW_BASE + len(OPS) - 1
    _d.CUSTOM_DVE_SPECS[name] = op.spec
    for ver in ("v3", "v4"):
        try:
            op.compile(ver)
        except ValueError as e:
            got = str(e).split(f"{ver}: ")[1].split(" ")[0]
            op.uops_sha[ver] = got
            op.compile(ver)
    return op


def _op_shrinkdx():
    # out = softshrink(in0 + in1, C1) - in0
    from concourse.dve_spec import Src0, Src1, C1, Zero, maxx, minn

    def body():
        z = Src0 + Src1
        c = minn(maxx(z, Zero - C1), C1)
        return (z - c) - Src0

    def ref(in0, in1, s0, s1, imm2):
        z = in0 + in1
        return (z - np.clip(z, -s1, s1)) - in0

    return _register_op("SHRINKDX", body, ref)


def _op_shrinkx():
    # out = softshrink(in0 + in1, C1)
    from concourse.dve_spec import Src0, Src1, C1, Zero, maxx, minn

    def body():
        z = Src0 + Src1
        c = minn(maxx(z, Zero - C1), C1)
        return z - c

    def ref(in0, in1, s0, s1, imm2):
        z = in0 + in1
        return z - np.clip(z, -s1, s1)

    return _register_op("SHRINKX", body, ref)


KA, KB = 128, 33

_CACHE = {}


def _host_constants(Drr, Dtheta):
    Drr = np.asarray(Drr, np.float32)
    Dtheta = np.asarray(Dtheta, np.float32)
    i = np.arange(T, dtype=np.float32)
    powr = (Drr[None, :] ** i[:, None]).astype(np.float32)
    sign = np.where(i[:, None] % 2 == 0, np.float32(1.0), np.float32(-1.0))
    ang = (i[:, None] * Dtheta[None, :]).astype(np.float32)
    dic = np.concatenate(
        [np.ones((T, 1), np.float32), powr * np.cos(ang),
         sign * powr * np.cos(ang), powr * np.sin(ang),
         sign * powr * np.sin(ang)], axis=1).astype(np.float32)
    G = np.sqrt(np.sum(dic * dic, axis=0, dtype=np.float32)).astype(np.float32)
    G = np.where(G == 0, np.sqrt(np.float32(T)), G).astype(np.float32)
    D = (dic / G).astype(np.float32)
    DtD = (D.T @ D).astype(np.float32)
    L = np.sqrt(np.sum(DtD.astype(np.float64) ** 2)).astype(np.float32)
    linv = np.float32(1.0) / L
    lambd = np.float32(LAMBD0 * linv)

    U = (-linv * D.T).astype(np.float32)
    M = (U @ D).astype(np.float32)
    A = (np.eye(K, dtype=np.float32) + M).astype(np.float32)
    f16 = lambda a: a.astype(np.float16)
    Uh, Dq, M16, A16 = f16(U), f16(D), f16(M), f16(A)

    hc = dict(lambd=lambd)
    # A16/M16 (and negated) blocks for rebuilds, [row-block, col-block]
    for nm, mat in (("a", A16), ("m", M16), ("an", -A16), ("mn", -M16)):
        hc[nm + "11"] = np.ascontiguousarray(mat[0:KA, 0:KA])
        hc[nm + "12"] = np.ascontiguousarray(mat[0:KA, KA:K])
        hc[nm + "21"] = np.ascontiguousarray(mat[KA:K, 0:KA])
        hc[nm + "22"] = np.ascontiguousarray(mat[KA:K, KA:K])

    # ---- static weight tiles ----
    I36 = np.eye(T, dtype=np.float16)
    # ODD-iter set (contract cur q-set only): W-regions rebuilt (1+tt)*A16
    wo_c1t1 = np.zeros((KA, KA), np.float16)
    wo_c1t2 = np.zeros((KA, 100), np.float16)
    wo_c1t2[:, 64:100] = Dq[:, 0:KA].T
    wo_c2t1 = np.zeros((100, KA), np.float16)
    wo_c2t1[64:100, :] = Uh[0:KA, :].T
    wo_c2t2 = np.zeros((100, 100), np.float16)
    wo_c2t2[0:33, 64:100] = Dq[:, KA:K].T
    wo_c2t2[64:100, 0:33] = Uh[KA:K, :].T
    wo_c2t2[64:100, 64:100] = I36
    # EVEN-iter cur-set: W-regions rebuilt stt(a,tt,m); same statics as odd
    we_c1t1 = wo_c1t1.copy()
    we_c1t2 = wo_c1t2.copy()
    we_c2t1 = wo_c2t1.copy()
    we_c2t2 = wo_c2t2.copy()
    # EVEN-iter prv-set: -W rebuilt; -D in macc cols; ZERO mh rows
    wf_c1t1 = np.zeros((KA, KA), np.float16)
    wf_c1t2 = np.zeros((KA, 100), np.float16)
    wf_c1t2[:, 64:100] = -Dq[:, 0:KA].T
    wf_c2t1 = np.zeros((100, KA), np.float16)
    wf_c2t2 = np.zeros((100, 100), np.float16)
    wf_c2t2[0:33, 64:100] = -Dq[:, KA:K].T
    tts = []
    t = np.float32(1.0)
    for _ in range(MAXITER):
        tn = (np.float32(1.0) + np.sqrt(np.float32(1.0) + np.float32(4.0) * t * t)) / np.float32(2.0)
        tts.append(np.float32((t - np.float32(1.0)) / tn))
        t = tn
    hc["tts"] = tts

    # Precompute per-iteration weight tiles on the host (DMA'd per iter on
    # device; kills all on-device rebuild ops). Odd iters (incl n=0) use
    # wo_* with W = (1+tt)*A16; even iters use we_* (tt*A16+M16 on cur) and
    # wf_* (negated on prv).
    f32 = np.float32
    n_odd = [n for n in range(MAXITER) if n % 2 == 1 or n == 0]
    n_even = [n for n in range(MAXITER) if n % 2 == 0 and n != 0]
    hc["odd_idx"] = {n: i for i, n in enumerate(n_odd)}
    hc["even_idx"] = {n: i for i, n in enumerate(n_even)}
    wo_all = {k: [] for k in ("c1t1", "c1t2", "c2t1", "c2t2")}
    we_all = {k: [] for k in ("c1t1", "c1t2", "c2t1", "c2t2")}
    wf_all = {k: [] for k in ("c1t1", "c1t2", "c2t1", "c2t2")}
    for n in n_odd:
        tt = f32(tts[n - 1]) if n > 0 else f32(0.0)
        Wn = ((1 + tt) * A16.astype(f32)).astype(np.float16)
        w1, w2, w3, w4 = wo_c1t1.copy(), wo_c1t2.copy(), wo_c2t1.copy(), wo_c2t2.copy()
        w1[:, :] = Wn[0:KA, 0:KA]
        w2[:, 0:33] = Wn[0:KA, KA:K]
        w3[0:33, :] = Wn[KA:K, 0:KA]
        w4[0:33, 0:33] = Wn[KA:K, KA:K]
        for k, w in zip(("c1t1", "c1t2", "c2t1", "c2t2"), (w1, w2, w3, w4)):
            wo_all[k].append(w)
    for n in n_even:
        tt = f32(tts[n - 1])
        Wn = (tt * A16.astype(f32) + M16.astype(f32)).astype(np.float16)
        w1, w2, w3, w4 = we_c1t1.copy(), we_c1t2.copy(), we_c2t1.copy(), we_c2t2.copy()
        w1[:, :] = Wn[0:KA, 0:KA]
        w2[:, 0:33] = Wn[0:KA, KA:K]
        w3[0:33, :] = Wn[KA:K, 0:KA]
        w4[0:33, 0:33] = Wn[KA:K, KA:K]
        for k, w in zip(("c1t1", "c1t2", "c2t1", "c2t2"), (w1, w2, w3, w4)):
            we_all[k].append(w)
        Wm = (-Wn.astype(f32)).astype(np.float16)
        v1, v2, v3, v4 = wf_c1t1.copy(), wf_c1t2.copy(), wf_c2t1.copy(), wf_c2t2.copy()
        v1[:, :] = Wm[0:KA, 0:KA]
        v2[:, 0:33] = Wm[0:KA, KA:K]
        v3[0:33, :] = Wm[KA:K, 0:KA]
        v4[0:33, 0:33] = Wm[KA:K, KA:K]
        for k, w in zip(("c1t1", "c1t2", "c2t1", "c2t2"), (v1, v2, v3, v4)):
            wf_all[k].append(w)
    for k in ("c1t1", "c1t2", "c2t1", "c2t2"):
        hc["wo_" + k] = np.ascontiguousarray(np.stack(wo_all[k]))
        hc["we_" + k] = np.ascontiguousarray(np.stack(we_all[k]))
        hc["wf_" + k] = np.ascontiguousarray(np.stack(wf_all[k]))
    return hc


def _build_bass(lambd, tts):
    import concourse.tile as tile
    from concourse import bacc, mybir
    dt = mybir.dt
    alu = mybir.AluOpType

    shdx = _op_shrinkdx()
    shx = _op_shrinkx()

    nc = bacc.Bacc("TRN2", target_bir_lowering=False, debug=False,
                   num_devices=N_CORES)
    n_odd = len([n for n in range(MAXITER) if n % 2 == 1 or n == 0])
    n_even = MAXITER - n_odd
    shp4 = {"c1t1": [KA, KA], "c1t2": [KA, 100],
            "c2t1": [100, KA], "c2t2": [100, 100]}
    wd = {}
    for pref, cnt in (("wo", n_odd), ("we", n_even), ("wf", n_even)):
        for k, s in shp4.items():
            nm = pref + "_" + k
            wd[nm] = nc.dram_tensor(nm, [cnt] + s, dt.float16,
                                    kind="ExternalInput").ap()
    ymh_d = nc.dram_tensor("ymh", [T, F], dt.float16, kind="ExternalInput").ap()
    xout_d = nc.dram_tensor("xout", [K, F], dt.float32, kind="ExternalOutput").ap()

    lam = float(lambd)
    odd_i, even_i = {}, {}
    oi = ei = 0
    for n in range(MAXITER):
        if n % 2 == 1 or n == 0:
            odd_i[n] = oi; oi += 1
        else:
            even_i[n] = ei; ei += 1

    with tile.TileContext(nc) as tc:
        with tc.tile_pool(name="stat", bufs=1) as stat, \
             tc.tile_pool(name="wb", bufs=2) as wbp, \
             tc.tile_pool(name="xt", bufs=2) as xtp, \
             tc.tile_pool(name="p1", bufs=2, space="PSUM") as p1p, \
             tc.tile_pool(name="p2", bufs=2, space="PSUM") as p2p:

            # q-sets: [set][tile]; set s written by op-A at iters n%2==s
            qs = []
            for s_ in range(2):
                q1 = stat.tile([KA, F], dt.float16, tag=f"q1_{s_}",
                               name=f"q1_{s_}")
                q2 = stat.tile([100, F], dt.float16, tag=f"q2_{s_}",
                               name=f"q2_{s_}")
                nc.vector.memset(q1[:], 0.0)
                nc.vector.memset(q2[:], 0.0)
                qs.append((q1, q2))
            xqa = stat.tile([KA, F], dt.float32, tag="xqa", name="xqa")
            xqb = stat.tile([KB, F], dt.float32, tag="xqb", name="xqb")
            nc.vector.memset(xqa[:], 0.0)
            nc.vector.memset(xqb[:], 0.0)
            # mh_0 into set-1's 2m tile (cur-set of iteration 0)
            nc.sync.dma_start(qs[1][1][64:100, :], ymh_d[:])

            for n in range(MAXITER):
                last = (n == MAXITER - 1)
                odd = (n % 2 == 1) or n == 0
                cur = qs[(n - 1) % 2]
                prv = qs[n % 2]
                out_set = qs[n % 2]

                # DMA-prefetch this iteration's weight tiles (rotating bufs)
                ws = {}
                if odd:
                    idx = odd_i[n]
                    for k, s in shp4.items():
                        w = wbp.tile(s, dt.float16, tag="wo_" + k,
                                     name="wo_" + k)
                        nc.sync.dma_start(w[:], wd["wo_" + k][idx])
                        ws["o_" + k] = w
                else:
                    idx = even_i[n]
                    for pref in ("we", "wf"):
                        for k, s in shp4.items():
                            w = wbp.tile(s, dt.float16, tag=pref + "_" + k,
                                         name=pref + "_" + k)
                            nc.sync.dma_start(w[:], wd[pref + "_" + k][idx])
                            ws[pref[1] + "_" + k] = w

                for gp in range(NG // 2):
                  pair = (2 * gp, 2 * gp + 1)
                  ptiles = {}
                  for g in pair:
                    ptiles[g] = (p1p.tile([KA, FG], dt.float32, tag="p1",
                                          name="p1"),
                                 p2p.tile([100, FG], dt.float32, tag="p2",
                                          name="p2"))
                  # weight-major across the group pair: each weight tile
                  # serves 4 consecutive 512-col streams before switching.
                  if odd:
                      seq = [("o_c1t1", 0, 0, True, False),
                             ("o_c2t1", 1, 0, False, True),
                             ("o_c1t2", 0, 1, True, False),
                             ("o_c2t2", 1, 1, False, True)]
                  else:
                      seq = [("e_c1t1", 0, 0, True, False),
                             ("f_c1t1", 2, 0, False, False),
                             ("e_c2t1", 1, 0, False, False),
                             ("f_c2t1", 3, 0, False, True),
                             ("e_c1t2", 0, 1, True, False),
                             ("f_c1t2", 2, 1, False, False),
                             ("e_c2t2", 1, 1, False, False),
                             ("f_c2t2", 3, 1, False, True)]
                  rsel = (cur[0], cur[1], prv[0], prv[1])
                  for wk, ri, pi, st_, sp_ in seq:
                      for g in pair:
                          for h in range(FG // 512):
                              s = slice(h * 512, (h + 1) * 512)
                              gs = slice(g * FG + h * 512,
                                         g * FG + (h + 1) * 512)
                              nc.tensor.matmul(ptiles[g][pi][:, s],
                                               ws[wk][:, :], rsel[ri][:, gs],
                                               start=st_, stop=sp_)
                  for g in pair:
                    cols = slice(g * FG, (g + 1) * FG)
                    p1, p2 = ptiles[g]
                    if not last:
                        nc.vector._custom_dve(shdx, out=out_set[0][:, cols],
                                              in0=xqa[:, cols], in1=p1[:, :],
                                              s1=lam)
                        nc.vector._custom_dve(shdx, out=out_set[1][0:33, cols],
                                              in0=xqb[:, cols],
                                              in1=p2[0:33, :], s1=lam)
                        nc.scalar.copy(out_set[1][64:100, cols], p2[64:100, :])
                        if odd and n > 0:
                            nc.gpsimd.tensor_tensor(xqa[:, cols], xqa[:, cols],
                                                    out_set[0][:, cols],
                                                    alu.add)
                            nc.gpsimd.tensor_tensor(xqb[:, cols], xqb[:, cols],
                                                    out_set[1][0:33, cols],
                                                    alu.add)
                    else:
                        xa = xtp.tile([KA, FG], dt.float32, tag="xa", name="xa")
                        xb = xtp.tile([KB, FG], dt.float32, tag="xb", name="xb")
                        nc.vector._custom_dve(shx, out=xa[:], in0=xqa[:, cols],
                                              in1=p1[:, :], s1=lam)
                        nc.vector._custom_dve(shx, out=xb[:], in0=xqb[:, cols],
                                              in1=p2[0:33, :], s1=lam)
                        nc.sync.dma_start(xout_d[0:KA, cols], xa[:])
                        nc.sync.dma_start(xout_d[KA:K, cols], xb[:])

    nc.compile()
    return nc


def _get_compiled(Drr, Dtheta):
    key = (np.asarray(Drr, np.float32).tobytes(),
           np.asarray(Dtheta, np.float32).tobytes())
    if key not in _CACHE:
        hc = _host_constants(Drr, Dtheta)
        nc = _build_bass(hc["lambd"], hc["tts"])
        _CACHE[key] = (nc, hc)
    return _CACHE[key]


def kernel(x, Drr, Dtheta):
    from concourse.bass_utils import run_bass_kernel_spmd
    x = np.asarray(x, np.float32)
    nc, hc = _get_compiled(Drr, Dtheta)

    wkeys = [k for k in hc if k.startswith(("wo_", "we_", "wf_"))]
    in_maps = []
    for c in range(N_CORES):
        xs = x[:, :, c * PLOC:(c + 1) * PLOC]
        yfull = np.ascontiguousarray(xs.transpose(1, 0, 2).reshape(T, F))
        m = {k: hc[k] for k in wkeys}
        m["ymh"] = (-yfull).astype(np.float16)
        in_maps.append(m)

    res = run_bass_kernel_spmd(nc, in_maps, core_ids=list(range(N_CORES)))
    global LAST_RESULTS
    LAST_RESULTS = res
    out = np.empty((B, K, P), np.float32)
    for c in range(N_CORES):
        xo = res.results[c]["xout"]
        out[:, :, c * PLOC:(c + 1) * PLOC] = (
            xo.reshape(K, B, PLOC).transpose(1, 0, 2))
    return out
